# revision 1
# baseline (speedup 1.0000x reference)
"""Skipgram negative-sampling loss kernel for 8 TRN2 NeuronCores.

Strategy: batch-data-parallel. The two embedding tables (1M x 128 f32) are
replicated into every core's DRAM; each core handles B/8 = 512 of the
(input, context, 5 negatives) id tuples. On-device per core:
  - one HWDGE DMA loads the 3584 ids (laid out [128, 28] by the host),
  - two SWDGE indirect DMAs gather the 512 u-rows and 3072 v-rows
    (512B/row descriptors - the efficient gather granularity),
  - six fused DVE tensor_tensor_reduce ops compute per-partition dot
    partial sums for (pos, neg0..neg4),
  - one PE matmul against a ones vector reduces over partitions,
  - the [1, 6] partial-sum vector is DMA'd out.
Host side only sums the 8 partial vectors (the unshard step for a
data-parallel reduction) and applies the final scalar logsigmoid formula.
"""

import numpy as np

V = 1_000_000
D = 128
B = 4096
K = 5
N_CORES = 8
BC = B // N_CORES  # 512 pairs per core
U_CHUNKS = BC // 128  # 4
V_CHUNKS = (K + 1) * BC // 128  # 24
N_GROUPS = K + 1  # pos + 5 negatives

_CACHE = {}


def _build(v_rows=V, dim=D, u_chunks=U_CHUNKS, v_chunks=V_CHUNKS):
    """Build + compile the per-core Bass program. Returns the Bacc object."""
    import concourse.bacc as bacc
    import concourse.tile as tile
    from concourse import bass, mybir

    n_groups = v_chunks // u_chunks
    nc = bacc.Bacc(
        "TRN2",
        target_bir_lowering=False,
        debug=False,
        enable_asserts=False,
        num_devices=N_CORES,
    )
    u_w = nc.dram_tensor("u_w", [v_rows, dim], mybir.dt.float32, kind="ExternalInput").ap()
    v_w = nc.dram_tensor("v_w", [v_rows, dim], mybir.dt.float32, kind="ExternalInput").ap()
    ids = nc.dram_tensor(
        "ids", [128, u_chunks + v_chunks], mybir.dt.int32, kind="ExternalInput"
    ).ap()
    out = nc.dram_tensor("out", [1, n_groups], mybir.dt.float32, kind="ExternalOutput").ap()

    with tile.TileContext(nc) as tc:
        with tc.tile_pool(name="p", bufs=1) as pool:
            ids_sb = pool.tile([128, u_chunks + v_chunks], mybir.dt.int32)
            nc.sync.dma_start(out=ids_sb[:], in_=ids[:])

            emb_u = pool.tile([128, u_chunks, dim], mybir.dt.float32)
            emb_v = pool.tile([128, v_chunks, dim], mybir.dt.float32)
            # The runtime DGE only handles [P, 1] offset columns correctly;
            # multi-column offset APs gather garbage on HW (sim accepts them).
            for j in range(u_chunks):
                nc.gpsimd.indirect_dma_start(
                    out=emb_u[:, j, :],
                    out_offset=None,
                    in_=u_w,
                    in_offset=bass.IndirectOffsetOnAxis(ap=ids_sb[:, j : j + 1], axis=0),
                )
            for j in range(v_chunks):
                nc.gpsimd.indirect_dma_start(
                    out=emb_v[:, j, :],
                    out_offset=None,
                    in_=v_w,
                    in_offset=bass.IndirectOffsetOnAxis(
                        ap=ids_sb[:, u_chunks + j : u_chunks + j + 1], axis=0
                    ),
                )

            prod = pool.tile([128, u_chunks, dim], mybir.dt.float32)
            cols = pool.tile([128, n_groups], mybir.dt.float32)
            u_flat = emb_u[:].rearrange("p c d -> p (c d)")
            for g in range(n_groups):
                vg = emb_v[:, g * u_chunks : (g + 1) * u_chunks, :].rearrange(
                    "p c d -> p (c d)"
                )
                nc.vector.tensor_tensor(
                    out=prod[:].rearrange("p c d -> p (c d)"),
                    in0=u_flat,
                    in1=vg,
                    op=mybir.AluOpType.mult,
                )
                nc.vector.reduce_sum(
                    out=cols[:, g : g + 1],
                    in_=prod[:].rearrange("p c d -> p (c d)"),
                    axis=mybir.AxisListType.X,
                )

            ones = pool.tile([128, 1], mybir.dt.float32)
            nc.vector.memset(ones[:], 1.0)
            with tc.tile_pool(name="ps", bufs=1, space="PSUM") as psum_pool:
                acc = psum_pool.tile([1, n_groups], mybir.dt.float32)
                nc.tensor.matmul(
                    out=acc[:], lhsT=ones[:], rhs=cols[:], start=True, stop=True
                )
                res = pool.tile([1, n_groups], mybir.dt.float32)
                nc.vector.tensor_copy(res[:], acc[:])
            nc.sync.dma_start(out=out, in_=res[:])

    nc.compile()
    return nc


def _get_nc():
    if "nc" not in _CACHE:
        _CACHE["nc"] = _build()
    return _CACHE["nc"]


def _prep_ids(input_words, context_words, neg_words):
    """Host-side shard + layout prep: per-core [128, 28] int32 id tiles.

    Column j of chunk layout: sbuf[p, j] = group_ids[(j % U_CHUNKS) * 128 + p]
    Groups along columns: [iw(4) | cw(4) | neg0(4) | ... | neg4(4)].
    """
    iw = np.ascontiguousarray(np.asarray(input_words).astype(np.int32))
    cw = np.ascontiguousarray(np.asarray(context_words).astype(np.int32))
    nw = np.ascontiguousarray(np.asarray(neg_words).astype(np.int32))
    per_core = []
    for c in range(N_CORES):
        sl = slice(c * BC, (c + 1) * BC)
        blocks = [iw[sl].reshape(U_CHUNKS, 128).T, cw[sl].reshape(U_CHUNKS, 128).T]
        for k in range(K):
            blocks.append(nw[k, sl].reshape(U_CHUNKS, 128).T)
        per_core.append(np.ascontiguousarray(np.hstack(blocks), dtype=np.int32))
    return per_core


def _log_sigmoid(x):
    x = np.asarray(x, dtype=np.float64)
    return np.where(x >= 0, -np.log1p(np.exp(-np.abs(x))), x - np.log1p(np.exp(x)))


def run_cores(inputs, trace=False, trace_kwargs=None):
    """Run the SPMD kernel on 8 cores; returns (per-core results, BassKernelResults)."""
    from concourse import bass_utils

    nc = _get_nc()
    u = np.ascontiguousarray(np.asarray(inputs["u_weight"], dtype=np.float32))
    v = np.ascontiguousarray(np.asarray(inputs["v_weight"], dtype=np.float32))
    ids_per_core = _prep_ids(
        inputs["input_words"], inputs["context_words"], inputs["neg_words"]
    )
    in_maps = [{"u_w": u, "v_w": v, "ids": ids_per_core[c]} for c in range(N_CORES)]
    res = bass_utils.run_bass_kernel_spmd(
        nc,
        in_maps,
        core_ids=list(range(N_CORES)),
        trace=trace,
        **(trace_kwargs or {}),
    )
    return res


def finish(partials):
    """Combine per-core [1,6] partial sums into the scalar loss."""
    s = np.sum(np.stack(partials, 0), axis=0).reshape(-1).astype(np.float64)
    pos_dot = s[0]
    neg_dots = s[1:]
    loss = -(_log_sigmoid(pos_dot) + np.sum(_log_sigmoid(-neg_dots)))
    return np.asarray(loss, dtype=np.float32)


def kernel(**inputs):
    res = run_cores(inputs, trace=False)
    partials = [res.results[c]["out"] for c in range(N_CORES)]
    return finish(partials)



# revision 2
# speedup vs baseline: 2.6850x; 2.6850x over previous
"""Skipgram negative-sampling loss kernel for 8 TRN2 NeuronCores.

Strategy: batch-data-parallel. The two embedding tables (1M x 128, stored as
bf16) are replicated into every core's DRAM; each core handles B/8 = 512 of
the (input, context, 5 negatives) id tuples. On-device per core:
  - one HWDGE DMA loads the 3584 ids (laid out [128, 28] by the host),
  - 28 SWDGE indirect DMAs gather the 512 u-rows and 3072 v-rows
    (256B bf16 rows; the DGE only handles [P, 1] offset columns),
  - six DVE tensor_tensor multiplies (bf16, 2x rate) + reduces compute
    per-partition dot partial sums for (pos, neg0..neg4),
  - the [128, 6] partial-sum tile is DMA'd out (no PE matmul tail).
Host side sums the 8 [128, 6] partials over partitions and cores and applies
the final scalar logsigmoid formula. bf16 keeps rel err ~1e-4, well under
the 2e-2 gate.
"""

import numpy as np
import ml_dtypes

V = 1_000_000
D = 128
B = 4096
K = 5
N_CORES = 8
BC = B // N_CORES  # 512 pairs per core
U_CHUNKS = BC // 128  # 4
V_CHUNKS = (K + 1) * BC // 128  # 24
N_GROUPS = K + 1  # pos + 5 negatives

_CACHE = {}


def _build(v_rows=V, dim=D, u_chunks=U_CHUNKS, v_chunks=V_CHUNKS):
    """Build + compile the per-core Bass program. Returns the Bacc object."""
    import concourse.bacc as bacc
    import concourse.tile as tile
    from concourse import bass, mybir

    n_groups = v_chunks // u_chunks
    nc = bacc.Bacc(
        "TRN2",
        target_bir_lowering=False,
        debug=False,
        enable_asserts=False,
        num_devices=N_CORES,
    )
    u_w = nc.dram_tensor("u_w", [v_rows, dim], mybir.dt.bfloat16, kind="ExternalInput").ap()
    v_w = nc.dram_tensor("v_w", [v_rows, dim], mybir.dt.bfloat16, kind="ExternalInput").ap()
    ids = nc.dram_tensor(
        "ids", [128, u_chunks + v_chunks], mybir.dt.int32, kind="ExternalInput"
    ).ap()
    out = nc.dram_tensor("out", [128, n_groups], mybir.dt.float32, kind="ExternalOutput").ap()

    with tile.TileContext(nc) as tc:
        with tc.tile_pool(name="p", bufs=1) as pool:
            ids_sb = pool.tile([128, u_chunks + v_chunks], mybir.dt.int32)
            nc.sync.dma_start(out=ids_sb[:], in_=ids[:])

            emb_u = pool.tile([128, u_chunks, dim], mybir.dt.bfloat16)
            emb_v = pool.tile([128, v_chunks, dim], mybir.dt.bfloat16)
            # The runtime DGE only handles [P, 1] offset columns correctly;
            # multi-column offset APs gather garbage on HW (sim accepts them).
            for j in range(u_chunks):
                nc.gpsimd.indirect_dma_start(
                    out=emb_u[:, j, :],
                    out_offset=None,
                    in_=u_w,
                    in_offset=bass.IndirectOffsetOnAxis(ap=ids_sb[:, j : j + 1], axis=0),
                )
            for j in range(v_chunks):
                nc.gpsimd.indirect_dma_start(
                    out=emb_v[:, j, :],
                    out_offset=None,
                    in_=v_w,
                    in_offset=bass.IndirectOffsetOnAxis(
                        ap=ids_sb[:, u_chunks + j : j + 1 + u_chunks], axis=0
                    ),
                )

            prod = pool.tile([128, u_chunks, dim], mybir.dt.bfloat16)
            cols = pool.tile([128, n_groups], mybir.dt.float32)
            u_flat = emb_u[:].rearrange("p c d -> p (c d)")
            for g in range(n_groups):
                vg = emb_v[:, g * u_chunks : (g + 1) * u_chunks, :].rearrange(
                    "p c d -> p (c d)"
                )
                nc.vector.tensor_tensor(
                    out=prod[:].rearrange("p c d -> p (c d)"),
                    in0=u_flat,
                    in1=vg,
                    op=mybir.AluOpType.mult,
                )
                nc.vector.reduce_sum(
                    out=cols[:, g : g + 1],
                    in_=prod[:].rearrange("p c d -> p (c d)"),
                    axis=mybir.AxisListType.X,
                )
            nc.sync.dma_start(out=out, in_=cols[:])

    nc.compile()
    return nc


def _get_nc():
    if "nc" not in _CACHE:
        _CACHE["nc"] = _build()
    return _CACHE["nc"]


def _prep_ids(input_words, context_words, neg_words):
    """Host-side shard + layout prep: per-core [128, 28] int32 id tiles.

    Column j of chunk layout: sbuf[p, j] = group_ids[(j % U_CHUNKS) * 128 + p]
    Groups along columns: [iw(4) | cw(4) | neg0(4) | ... | neg4(4)].
    """
    iw = np.ascontiguousarray(np.asarray(input_words).astype(np.int32))
    cw = np.ascontiguousarray(np.asarray(context_words).astype(np.int32))
    nw = np.ascontiguousarray(np.asarray(neg_words).astype(np.int32))
    per_core = []
    for c in range(N_CORES):
        sl = slice(c * BC, (c + 1) * BC)
        blocks = [iw[sl].reshape(U_CHUNKS, 128).T, cw[sl].reshape(U_CHUNKS, 128).T]
        for k in range(K):
            blocks.append(nw[k, sl].reshape(U_CHUNKS, 128).T)
        per_core.append(np.ascontiguousarray(np.hstack(blocks), dtype=np.int32))
    return per_core


def _log_sigmoid(x):
    x = np.asarray(x, dtype=np.float64)
    return np.where(x >= 0, -np.log1p(np.exp(-np.abs(x))), x - np.log1p(np.exp(x)))


def run_cores(inputs, trace=False, trace_kwargs=None):
    """Run the SPMD kernel on 8 cores; returns BassKernelResults."""
    from concourse import bass_utils

    nc = _get_nc()
    u = np.asarray(inputs["u_weight"], dtype=np.float32).astype(ml_dtypes.bfloat16)
    v = np.asarray(inputs["v_weight"], dtype=np.float32).astype(ml_dtypes.bfloat16)
    ids_per_core = _prep_ids(
        inputs["input_words"], inputs["context_words"], inputs["neg_words"]
    )
    in_maps = [{"u_w": u, "v_w": v, "ids": ids_per_core[c]} for c in range(N_CORES)]
    res = bass_utils.run_bass_kernel_spmd(
        nc,
        in_maps,
        core_ids=list(range(N_CORES)),
        trace=trace,
        **(trace_kwargs or {}),
    )
    return res


def finish(partials):
    """Combine per-core [128, 6] partial sums into the scalar loss."""
    s = np.zeros(N_GROUPS, np.float64)
    for p in partials:
        s += np.asarray(p, dtype=np.float64).reshape(128, N_GROUPS).sum(axis=0)
    pos_dot = s[0]
    neg_dots = s[1:]
    loss = -(_log_sigmoid(pos_dot) + np.sum(_log_sigmoid(-neg_dots)))
    return np.asarray(loss, dtype=np.float32)


def kernel(**inputs):
    res = run_cores(inputs, trace=False)
    partials = [res.results[c]["out"] for c in range(N_CORES)]
    return finish(partials)


# revision 7
# speedup vs baseline: 2.7721x; 1.0324x over previous
"""Skipgram negative-sampling loss kernel for 8 TRN2 NeuronCores.

Strategy: batch-data-parallel. The two embedding tables (1M x 128, stored as
bf16) are replicated into every core's DRAM; each core handles B/8 = 512 of
the (input, context, 5 negatives) id tuples. On-device per core:
  - one HWDGE DMA loads the 3584 ids (laid out [128, 28] by the host),
  - 28 SWDGE indirect DMAs gather the 512 u-rows and 3072 v-rows
    (256B bf16 rows; the DGE only handles [P, 1] offset columns),
  - six DVE tensor_tensor multiplies (bf16, 2x rate) + reduces compute
    per-partition dot partial sums for (pos, neg0..neg4),
  - the [128, 6] partial-sum tile is DMA'd out (no PE matmul tail).
Host side sums the 8 [128, 6] partials over partitions and cores and applies
the final scalar logsigmoid formula. bf16 keeps rel err ~1e-4, well under
the 2e-2 gate.
"""

import numpy as np
import ml_dtypes

V = 1_000_000
D = 128
B = 4096
K = 5
N_CORES = 8
BC = B // N_CORES  # 512 pairs per core
U_CHUNKS = BC // 128  # 4
V_CHUNKS = (K + 1) * BC // 128  # 24
N_GROUPS = K + 1  # pos + 5 negatives

_CACHE = {}


def _build(v_rows=V, dim=D, u_chunks=U_CHUNKS, v_chunks=V_CHUNKS):
    """Build + compile the per-core Bass program. Returns the Bacc object."""
    import concourse.bacc as bacc
    import concourse.tile as tile
    from concourse import bass, mybir

    n_groups = v_chunks // u_chunks
    nc = bacc.Bacc(
        "TRN2",
        target_bir_lowering=False,
        debug=False,
        enable_asserts=False,
        num_devices=N_CORES,
    )
    u_w = nc.dram_tensor("u_w", [v_rows, dim], mybir.dt.bfloat16, kind="ExternalInput").ap()
    v_w = nc.dram_tensor("v_w", [v_rows, dim], mybir.dt.bfloat16, kind="ExternalInput").ap()
    ids_u = nc.dram_tensor("ids_u", [128, u_chunks], mybir.dt.int32, kind="ExternalInput").ap()
    ids_v = nc.dram_tensor("ids_v", [128, v_chunks], mybir.dt.int32, kind="ExternalInput").ap()
    out = nc.dram_tensor("out", [128, n_groups], mybir.dt.float32, kind="ExternalOutput").ap()

    with tile.TileContext(nc) as tc:
        with tc.tile_pool(name="p", bufs=1) as pool:
            # Split id loads: the small u-id DMA (2KB) completes first so the
            # first gather starts earlier; the v-id DMA overlaps the u-gathers.
            idu_sb = pool.tile([128, u_chunks], mybir.dt.int32)
            idv_sb = pool.tile([128, v_chunks], mybir.dt.int32)
            nc.sync.dma_start(out=idu_sb[:], in_=ids_u[:])
            nc.sync.dma_start(out=idv_sb[:], in_=ids_v[:])

            emb_u = pool.tile([128, u_chunks, dim], mybir.dt.bfloat16)
            emb_v = pool.tile([128, v_chunks, dim], mybir.dt.bfloat16)
            # The runtime DGE only handles [P, 1] offset columns correctly;
            # multi-column offset APs gather garbage on HW (sim accepts them).
            for j in range(u_chunks):
                nc.gpsimd.indirect_dma_start(
                    out=emb_u[:, j, :],
                    out_offset=None,
                    in_=u_w,
                    in_offset=bass.IndirectOffsetOnAxis(ap=idu_sb[:, j : j + 1], axis=0),
                )
            for j in range(v_chunks):
                nc.gpsimd.indirect_dma_start(
                    out=emb_v[:, j, :],
                    out_offset=None,
                    in_=v_w,
                    in_offset=bass.IndirectOffsetOnAxis(
                        ap=idv_sb[:, j : j + 1], axis=0
                    ),
                )

            # Per-column multiplies: each mult only waits for its own gathered
            # column, so after the LAST gather completes just one 128-column
            # mult + one reduce remain on the tail.
            prod = pool.tile([128, v_chunks, dim], mybir.dt.bfloat16)
            cols = pool.tile([128, n_groups], mybir.dt.float32)
            for g in range(n_groups):
                for c in range(u_chunks):
                    j = g * u_chunks + c
                    nc.vector.tensor_tensor(
                        out=prod[:, j, :],
                        in0=emb_u[:, c, :],
                        in1=emb_v[:, j, :],
                        op=mybir.AluOpType.mult,
                    )
                nc.vector.reduce_sum(
                    out=cols[:, g : g + 1],
                    in_=prod[:, g * u_chunks : (g + 1) * u_chunks, :].rearrange(
                        "p c d -> p (c d)"
                    ),
                    axis=mybir.AxisListType.X,
                )
            nc.sync.dma_start(out=out, in_=cols[:])

    nc.compile()
    return nc


def _get_nc():
    if "nc" not in _CACHE:
        _CACHE["nc"] = _build()
    return _CACHE["nc"]


def _prep_ids(input_words, context_words, neg_words):
    """Host-side shard + layout prep: per-core [128, 28] int32 id tiles.

    Column j of chunk layout: sbuf[p, j] = group_ids[(j % U_CHUNKS) * 128 + p]
    Groups along columns: [iw(4) | cw(4) | neg0(4) | ... | neg4(4)].
    """
    iw = np.ascontiguousarray(np.asarray(input_words).astype(np.int32))
    cw = np.ascontiguousarray(np.asarray(context_words).astype(np.int32))
    nw = np.ascontiguousarray(np.asarray(neg_words).astype(np.int32))
    per_core = []
    for c in range(N_CORES):
        sl = slice(c * BC, (c + 1) * BC)
        ids_u = np.ascontiguousarray(iw[sl].reshape(U_CHUNKS, 128).T, dtype=np.int32)
        blocks = [cw[sl].reshape(U_CHUNKS, 128).T]
        for k in range(K):
            blocks.append(nw[k, sl].reshape(U_CHUNKS, 128).T)
        ids_v = np.ascontiguousarray(np.hstack(blocks), dtype=np.int32)
        per_core.append((ids_u, ids_v))
    return per_core


def _log_sigmoid(x):
    x = np.asarray(x, dtype=np.float64)
    return np.where(x >= 0, -np.log1p(np.exp(-np.abs(x))), x - np.log1p(np.exp(x)))


def run_cores(inputs, trace=False, trace_kwargs=None):
    """Run the SPMD kernel on 8 cores; returns BassKernelResults."""
    from concourse import bass_utils

    nc = _get_nc()
    u = np.asarray(inputs["u_weight"], dtype=np.float32).astype(ml_dtypes.bfloat16)
    v = np.asarray(inputs["v_weight"], dtype=np.float32).astype(ml_dtypes.bfloat16)
    ids_per_core = _prep_ids(
        inputs["input_words"], inputs["context_words"], inputs["neg_words"]
    )
    in_maps = [
        {"u_w": u, "v_w": v, "ids_u": ids_per_core[c][0], "ids_v": ids_per_core[c][1]}
        for c in range(N_CORES)
    ]
    res = bass_utils.run_bass_kernel_spmd(
        nc,
        in_maps,
        core_ids=list(range(N_CORES)),
        trace=trace,
        **(trace_kwargs or {}),
    )
    return res


def finish(partials):
    """Combine per-core [128, 6] partial sums into the scalar loss."""
    s = np.zeros(N_GROUPS, np.float64)
    for p in partials:
        s += np.asarray(p, dtype=np.float64).reshape(128, N_GROUPS).sum(axis=0)
    pos_dot = s[0]
    neg_dots = s[1:]
    loss = -(_log_sigmoid(pos_dot) + np.sum(_log_sigmoid(-neg_dots)))
    return np.asarray(loss, dtype=np.float32)


def kernel(**inputs):
    res = run_cores(inputs, trace=False)
    partials = [res.results[c]["out"] for c in range(N_CORES)]
    return finish(partials)
